# revision 1
# baseline (speedup 1.0000x reference)
"""MaxPool3d (kernel=3, stride=2, padding=1) on Trainium2, 8 NeuronCores.

Input  x: (2, 32, 128, 128, 128) f32  ->  Output: (2, 32, 64, 64, 64) f32.

Sharding: the 64 (b, c) slices are data-parallel; each of the 8 cores gets 8
slices, processed as 4 slice-pairs (a pair packs 2 slices into the 128 SBUF
partitions).

Per-core algorithm (separable max pooling W -> H -> D):
  - Load each slice-pair's depth rows into two "parity slabs": even-d rows in
    xE (partition 64*s + d/2), odd-d rows in xO. This makes the final D-axis
    pooling a partition-aligned elementwise max between slabs.
  - W pool (free axis): F = max(x[..., 0::2], x[..., 1::2]);
    F[..., 1:] = max(F[..., 1:], x[..., 1:126:2]).
  - H pool (free axis): G = max(F[:, 0::2], F[:, 1::2]);
    G[:, 1:] = max(G[:, 1:], F[:, 1:126:2]).  (slab E writes straight into
    the output tile Et)
  - D pool (partition axis): Et = max(Et, G_O); the 2*od-1 term comes from a
    partition-shifted SBUF->SBUF DMA copy of G_O plus one more max.

DMA notes: loads alternate between the two HWDGE rings (nc.sync / nc.scalar)
to halve per-ring FIFO serialization; each load moves a full slice-pair
chunk (2 MiB) in one call.
"""

import os
import sys

sys.path.insert(0, "/opt/trn_rl_repo")

import numpy as np

# Shapes (hardcoded per problem spec)
B, C, D, H, W = 2, 32, 128, 128, 128
OD, OH, OW = 64, 64, 64
N_CORES = 8
SLICES_PER_CORE = (B * C) // N_CORES  # 8
PAIRS = SLICES_PER_CORE // 2  # 4
HC = 32  # max h rows per load chunk (tile size)
# ramp-friendly schedule: small first chunks (pair 0 only) so DVE starts early
CHUNK_SIZES_RAMP = [8, 24, 32, 32, 32]
CHUNK_SIZES_STEADY = [32, 32, 32, 32]
assert sum(CHUNK_SIZES_RAMP) == H and max(CHUNK_SIZES_RAMP) == HC
assert sum(CHUNK_SIZES_STEADY) == H

_cache = {}


def _build():
    import concourse.mybir as mybir
    from concourse import bacc
    from concourse.tile import TileContext

    f32 = mybir.dt.float32
    nc = bacc.Bacc()
    x_ext = nc.declare_dram_parameter(
        "x_shard", [SLICES_PER_CORE, D, H, W], f32, isOutput=False
    )
    y_ext = nc.declare_dram_parameter(
        "y_shard", [SLICES_PER_CORE, OD, OH, OW], f32, isOutput=True
    )

    with TileContext(nc) as tc:
        with (
            tc.tile_pool(name="xpool", bufs=3) as xpool,
            tc.tile_pool(name="fpool", bufs=3) as fpool,
            tc.tile_pool(name="gpool", bufs=3) as gpool,
            tc.tile_pool(name="opool", bufs=2) as opool,
        ):
            dma_rr = [0]

            def load_engine():
                # alternate between the two HWDGE rings
                dma_rr[0] ^= 1
                return nc.sync if dma_rr[0] else nc.scalar

            for p in range(PAIRS):
                s0 = 2 * p
                # H pool: slab E accumulates into Et (global rows); slab O
                # goes to a per-chunk Go tile (local rows)
                Et = opool.tile([128, OH, OW], f32, name="Et", tag="Et")
                Fprev = {0: None, 1: None}
                h0 = 0
                sizes = CHUNK_SIZES_RAMP if p == 0 else CHUNK_SIZES_STEADY
                for c, hc in enumerate(sizes):
                    oh0 = h0 // 2
                    ohc = hc // 2
                    ohr = slice(oh0, oh0 + ohc)
                    Go = None
                    for par, name in ((0, "E"), (1, "O")):
                        xt = xpool.tile(
                            [128, HC, W], f32, name=f"x{name}", tag=f"x{name}"
                        )
                        load_engine().dma_start(
                            out=xt[:, 0:hc, :],
                            in_=x_ext[s0 : s0 + 2, par : D : 2, h0 : h0 + hc, :],
                        )
                        # ---- W pool into per-chunk F tile ----
                        Ft = fpool.tile(
                            [128, HC, OW], f32, name=f"F{name}", tag=f"F{name}"
                        )
                        nc.vector.tensor_max(
                            out=Ft[:, 0:hc, :],
                            in0=xt[:, 0:hc, 0:W:2],
                            in1=xt[:, 0:hc, 1:W:2],
                        )
                        nc.vector.tensor_max(
                            out=Ft[:, 0:hc, 1:OW],
                            in0=Ft[:, 0:hc, 1:OW],
                            in1=xt[:, 0:hc, 1 : W - 2 : 2],
                        )
                        # ---- H pool rows of this chunk ----
                        if par == 0:
                            Gt, g0 = Et, oh0
                        else:
                            Go = gpool.tile(
                                [128, HC // 2, OW], f32, name="Go", tag="Go"
                            )
                            Gt, g0 = Go, 0
                        nc.vector.tensor_max(
                            out=Gt[:, g0 : g0 + ohc, :],
                            in0=Ft[:, 0:hc:2, :],
                            in1=Ft[:, 1:hc:2, :],
                        )
                        nc.vector.tensor_max(
                            out=Gt[:, g0 + 1 : g0 + ohc, :],
                            in0=Gt[:, g0 + 1 : g0 + ohc, :],
                            in1=Ft[:, 1 : hc - 2 : 2, :],
                        )
                        if c > 0:
                            # boundary row: h = 2*oh0 - 1 = prev chunk's last row
                            nc.vector.tensor_max(
                                out=Gt[:, g0 : g0 + 1, :],
                                in0=Gt[:, g0 : g0 + 1, :],
                                in1=Fprev[par],
                            )
                        Fprev[par] = Ft[:, hc - 1 : hc, :]

                    # ---- incremental D pool on this chunk's finalized rows ----
                    # partition-shift of Go rows (d axis); rows 0/64 get values
                    # already folded into Et (idempotent under max).
                    Gs = fpool.tile([128, HC // 2, OW], f32, name="Gs", tag="Gs")
                    nc.scalar.dma_start(
                        out=Gs[1:64, 0:ohc, :], in_=Go[0:63, 0:ohc, :]
                    )
                    nc.scalar.dma_start(
                        out=Gs[65:128, 0:ohc, :], in_=Go[64:127, 0:ohc, :]
                    )
                    nc.sync.dma_start(
                        out=Gs[0:65:64, 0:ohc, :], in_=Go[0:65:64, 0:ohc, :]
                    )
                    nc.vector.tensor_max(
                        out=Et[:, ohr, :], in0=Et[:, ohr, :], in1=Go[:, 0:ohc, :]
                    )
                    nc.vector.tensor_max(
                        out=Et[:, ohr, :], in0=Et[:, ohr, :], in1=Gs[:, 0:ohc, :]
                    )
                    # ---- store this chunk's finalized output rows ----
                    nc.sync.dma_start(
                        out=y_ext[s0 : s0 + 2, :, ohr, :], in_=Et[:, ohr, :]
                    )
                    h0 += hc
    nc.compile()
    return nc


def _get_nc():
    if "nc" not in _cache:
        _cache["nc"] = _build()
    return _cache["nc"]


def run(x: np.ndarray, **spmd_kwargs):
    """Run the SPMD kernel; returns the BassKernelResults (for tracing)."""
    from concourse.bass_utils import run_bass_kernel_spmd

    nc = _get_nc()
    xs = np.ascontiguousarray(x, dtype=np.float32).reshape(B * C, D, H, W)
    in_maps = [
        {"x_shard": np.ascontiguousarray(xs[SLICES_PER_CORE * i : SLICES_PER_CORE * (i + 1)])}
        for i in range(N_CORES)
    ]
    return run_bass_kernel_spmd(nc, in_maps, list(range(N_CORES)), **spmd_kwargs)


def kernel(x: np.ndarray) -> np.ndarray:
    res = run(x)
    out = np.stack([res.results[i]["y_shard"] for i in range(N_CORES)])
    return out.reshape(B, C, OD, OH, OW)

